# revision 17
# baseline (speedup 1.0000x reference)
"""Trainium2 Bass kernel for nn_CausalSelfAttention (B=2, T=2048, C=1024, 16 heads).

Sharding: 8 cores = 2 batches x 4 head-groups (4 heads each). Each core:
  - computes Q^T/K^T ([d,t] layout) and V ([t,d]) for its heads from x^T
    (host pre-transposes x and pre-packs the weight slices in SBUF layout),
  - runs causal flash attention: S^T ([k,q]) via PE, exp on ScalarE with the
    1/sqrt(d) scale fused, then P@V with P^T as the *stationary* operand so
    the PV matmul runs at full PE rate ([q,d] output, 128-partition out),
    softmax denominators from a ones-column in V,
  - normalizes O in [q,d] layout (per-partition scalars), transposes O back
    to [d,q] via the DMA XBAR (off the PE critical path),
  - projects through its W_out row-slice producing a partial [T, C] output.
Out-projection blocks are interleaved into the attention loop as PE fillers;
a single set of tile pools is used throughout (no mid-kernel pool barriers).
Host sums the 4 tensor-parallel partials per batch (the "all-reduce") and
adds b_out.

Matmul inputs are bf16 (fp32 accumulation in PSUM).
"""
import sys

if '/opt/trn_rl_repo' not in sys.path:
    sys.path.insert(0, '/opt/trn_rl_repo')

import numpy as np
import ml_dtypes

B, T, C = 2, 2048, 1024
N_HEAD = 16
D = 64
P = 128
N_CORES = 8
GROUPS = N_CORES // B            # 4 tensor-parallel groups per batch
HPC = N_HEAD // GROUPS           # 4 heads per core
DH = HPC * D                     # 256 head dims per core
KO = C // P                      # 8 contraction subtiles for projections
NQB = T // 512                   # 4 q blocks of 512
SCALE = 1.0 / np.sqrt(D)
N_WARM = 12                       # PE warm-up matmuls while first loads land

_CACHE = {}


def _build():
    import concourse.mybir as mybir
    import concourse.tile as tile
    from concourse import bacc

    f32 = mybir.dt.float32
    bf16 = mybir.dt.bfloat16
    f16 = mybir.dt.float16

    nc = bacc.Bacc("TRN2", target_bir_lowering=False, debug=False,
                   num_devices=N_CORES)

    xt_d = nc.dram_tensor("xt", [C, T], bf16, kind="ExternalInput")
    wq_d = nc.dram_tensor("wq", [P, KO, DH], bf16, kind="ExternalInput")
    wk_d = nc.dram_tensor("wk", [P, KO, DH], bf16, kind="ExternalInput")
    wv_d = nc.dram_tensor("wv", [P, KO, DH], bf16, kind="ExternalInput")
    wo_d = nc.dram_tensor("wo", [P, 2, C], bf16, kind="ExternalInput")
    bq_d = nc.dram_tensor("bq", [P, 2], f32, kind="ExternalInput")
    bk_d = nc.dram_tensor("bk", [P, 2], f32, kind="ExternalInput")
    bv_d = nc.dram_tensor("bv", [1, DH], f32, kind="ExternalInput")
    tri_d = nc.dram_tensor("tri", [P, P], bf16, kind="ExternalInput")
    out_d = nc.dram_tensor("out", [T, C], f16, kind="ExternalOutput")

    EXP = mybir.ActivationFunctionType.Exp
    pls = [slice(0, D), slice(D, 2 * D)]

    with tile.TileContext(nc) as tc:
        with (
            tc.tile_pool(name="pp", bufs=1) as pp,
            tc.tile_pool(name="wB", bufs=32) as wB,
            tc.tile_pool(name="wS", bufs=4) as wS,
            tc.tile_pool(name="wO", bufs=4) as wO,
            tc.tile_pool(name="psS", bufs=2, space="PSUM") as psS,
            tc.tile_pool(name="psU", bufs=4, space="PSUM") as psU,
        ):
            xts = [pp.tile([P, KO, 512], bf16, tag=f"xt{q}", name=f"xt{q}")
                   for q in range(4)]
            wqs = pp.tile([P, KO, DH], bf16, tag="wqs")
            wks = pp.tile([P, KO, DH], bf16, tag="wks")
            wvs = pp.tile([P, KO, DH], bf16, tag="wvs")
            wos = pp.tile([P, 2, C], bf16, tag="wos")
            qts = [[pp.tile([P, 512], bf16, tag=f"qt{s}_{q}", name=f"qt{s}_{q}")
                    for q in range(4)] for s in range(2)]
            kts = [[pp.tile([P, 512], bf16, tag=f"kt{s}_{q}", name=f"kt{s}_{q}")
                    for q in range(4)] for s in range(2)]
            vos = [pp.tile([P, 4, HPC, D + 1], bf16, tag=f"vo{q}",
                           name=f"vo{q}") for q in range(4)]
            ots = [[pp.tile([P, 512], bf16, tag=f"ot{j}_{hs}",
                            name=f"ot{j}_{hs}") for hs in range(2)]
                   for j in range(NQB)]
            bqs = pp.tile([P, 2], f32, tag="bqs")
            bks = pp.tile([P, 2], f32, tag="bks")
            bvrow = pp.tile([1, DH], f32, tag="bvrow")
            bvb = pp.tile([P, DH], f32, tag="bvb")
            trib = pp.tile([P, P], bf16, tag="trib")
            warm = pp.tile([P, 512], bf16, tag="warm")

            # PE warm-up: junk matmuls on a zeroed tile keep the PE p-state
            # ramp going while the first input DMAs land.
            nc.vector.memset(warm[:], 0.0)
            for w in range(N_WARM):
                pw = psU.tile([P, 512], f32, tag="ps1", name=f"pw{w}")
                nc.tensor.matmul(pw[:], warm[:, 0:P], warm[:],
                                 start=True, stop=True)

            # ---- loads (SP + ACT HWDGE queues in parallel at startup) ----
            xt_r = xt_d.rearrange("(ko p) t -> p ko t", p=P)
            nc.sync.dma_start(wqs[:, :, 0:P], wq_d[:, :, 0:P])
            nc.scalar.dma_start(xts[0][:, :, 0:256], xt_r[:, :, 0:256])
            nc.sync.dma_start(bqs[:], bq_d[:])
            nc.sync.dma_start(wqs[:, :, P:2 * P], wq_d[:, :, P:2 * P])
            nc.scalar.dma_start(xts[0][:, :, 256:512], xt_r[:, :, 256:512])
            nc.sync.dma_start(bks[:], bk_d[:])
            nc.sync.dma_start(trib[:], tri_d[:])
            nc.sync.dma_start(bvrow[:], bv_d[:])
            nc.sync.dma_start(wks[:], wk_d[:])
            nc.sync.dma_start(xts[1][:], xt_r[:, :, 512:1024])
            nc.sync.dma_start(wvs[:], wv_d[:])
            nc.sync.dma_start(xts[2][:], xt_r[:, :, 1024:1536])
            nc.sync.dma_start(xts[3][:], xt_r[:, :, 1536:2048])
            nc.sync.dma_start(wos[:], wo_d[:])

            nc.gpsimd.partition_broadcast(bvb[:, :], bvrow[0:1, :])
            for q in range(4):
                nc.vector.memset(vos[q][:, :, :, D:D + 1], 1.0)
            # trigger the exp ACT-table load early
            scr = pp.tile([1, 1], f32, tag="scr")
            nc.scalar.activation(scr[0:1, 0:1], trib[0:1, 0:1], EXP)

            # ---- emit helpers (each *_items returns a list of thunks; one
            # thunk = one contiguous chunk of PE work) ----
            def qk_items(q, chunks):
                items = []
                for wsb, dsts, bias in ((wqs, qts, bqs), (wks, kts, bks)):
                    for s_ in range(2):
                        for (lo, hi) in chunks:
                            def it(wsb=wsb, dsts=dsts, bias=bias, s_=s_,
                                   lo=lo, hi=hi):
                                w_ = hi - lo
                                pq = psU.tile([P, 512], f32, tag="ps1",
                                              name=f"pq{q}_{s_}_{lo}")
                                for ko in range(KO):
                                    nc.tensor.matmul(
                                        pq[:, 0:w_],
                                        wsb[:, ko, s_ * P:(s_ + 1) * P],
                                        xts[q][:, ko, lo:hi],
                                        start=(ko == 0), stop=(ko == KO - 1))
                                nc.vector.tensor_scalar_add(
                                    dsts[s_][q][:, lo:hi], pq[:, 0:w_],
                                    bias[:, s_:s_ + 1])
                            items.append(it)
                return items

            def v_items(q):
                items = []
                for it_ in range(4):
                    def it(it_=it_):
                        pv = psU.tile([P, 512], f32, tag="ps1",
                                      name=f"pv{q}_{it_}")
                        for ko in range(KO):
                            nc.tensor.matmul(
                                pv[:, 0:DH],
                                xts[q][:, ko, it_ * P:(it_ + 1) * P],
                                wvs[:, ko, :],
                                start=(ko == 0), stop=(ko == KO - 1))
                        nc.vector.tensor_tensor(
                            vos[q][:, it_, :, 0:D],
                            pv[:, 0:DH].rearrange("p (h d) -> p h d", h=HPC),
                            bvb.rearrange("p (h d) -> p h d", h=HPC),
                            mybir.AluOpType.add)
                    items.append(it)
                return items

            def emit_s_tile(q, hs, i, pts):
                off = max(0, P * i - 512 * q)
                sp = psS.tile([P, 2, 512], f32, tag="sp",
                              name=f"sp{q}_{hs}_{i}")
                pt = wB.tile([P, 2, 512], bf16, tag="pt",
                             name=f"pt{q}_{hs}_{i}")
                for u in range(2):
                    nc.tensor.matmul(
                        sp[:, u, off:512],
                        kts[hs][i // 4][pls[u],
                                        (i % 4) * P:(i % 4 + 1) * P],
                        qts[hs][q][pls[u], off:512],
                        start=True, stop=True)
                nc.scalar.activation(pt[:, :, off:512],
                                     sp[:, :, off:512],
                                     EXP, scale=float(SCALE))
                if P * i >= 512 * q:  # diagonal triangle
                    for u in range(2):
                        nc.vector.tensor_mul(
                            pt[:, u, off:off + P],
                            pt[:, u, off:off + P], trib[:])
                pts.append(pt)

            def emit_pv(q, hs, pts, qcs):
                for qc in qcs:
                    qt = 4 * q + qc
                    # one PSUM bank per head (zero-region = 2KB: only one
                    # accumulation group may live in a bank at a time)
                    po = [psU.tile([P, 512], f32, tag="ps1",
                                   name=f"po{q}_{hs}_{qc}_{u}")
                          for u in range(2)]
                    for u in range(2):
                        for kt in range(qt + 1):
                            nc.tensor.matmul(
                                po[u][:, 0:D + 1],
                                pts[kt][:, u, qc * P:(qc + 1) * P],
                                vos[kt // 4][:, kt % 4, 2 * hs + u, :],
                                start=(kt == 0), stop=(kt == qt))
                    rd = wS.tile([P, 2], f32, tag="rd",
                                 name=f"rd{q}_{hs}_{qc}")
                    osb = wS.tile([P, 2, D], bf16, tag="osb",
                                  name=f"osb{q}_{hs}_{qc}")
                    for u in range(2):
                        nc.vector.reciprocal_approx_fast(
                            rd[:, u:u + 1], po[u][:, D:D + 1])
                        nc.vector.tensor_scalar_mul(
                            osb[:, u, :], po[u][:, 0:D], rd[:, u:u + 1])
                    nc.sync.dma_start(
                        ots[q][hs][:, qc * P:(qc + 1) * P], osb[:],
                        transpose=True)

            def c_items(jj, mos):
                # one [P, C] staging tile + ONE store per row-block, issued
                # through the software DGE on the idle GpSimd engine so the
                # stores never contend with loads/transposes on HWDGE
                items = []
                obs = {}
                for mo in mos:
                    for n in range(2):
                        def it(mo=mo, n=n):
                            m = 4 * jj + mo
                            pc = psU.tile([P, 512], f32, tag="ps1",
                                          name=f"pc{jj}_{mo}_{n}")
                            for s in range(2):
                                nc.tensor.matmul(
                                    pc[:],
                                    ots[jj][s][:, mo * P:(mo + 1) * P],
                                    wos[:, s, n * 512:(n + 1) * 512],
                                    start=(s == 0), stop=(s == 1))
                            if n == 0:
                                obs[mo] = wO.tile([P, C], f16, tag="ob",
                                                  name=f"ob{jj}_{mo}")
                            ob = obs[mo]
                            nc.vector.tensor_copy(
                                ob[:, n * 512:(n + 1) * 512], pc[:])
                            if n == 1:
                                nc.gpsimd.dma_start(
                                    out_d[m * P:(m + 1) * P, :], ob[:])
                        items.append(it)
                return items

            # ---- main pipeline ----
            # Per iteration q: spread the S i-tiles (whose exps are the ACT
            # bottleneck) among filler PE work (prev block's out-projection,
            # part of the next quarter's projections); the rest of the A
            # segment sits between PV(q,0) and PV(q,1) so the second head
            # pair's exps can drain before PV(q,1) consumes them.
            def spread(tiles, fillers):
                # emit S tiles with fillers distributed evenly between them
                nS, nF = len(tiles), len(fillers)
                fi = 0
                for k, t in enumerate(tiles, 1):
                    t()
                    while fi < (k * nF) // nS:
                        fillers[fi]()
                        fi += 1

            for it in qk_items(0, [(0, 256), (256, 512)]) + v_items(0):
                it()
            for q in range(4):
                fillA = (qk_items(q + 1, [(0, 512)]) + v_items(q + 1)
                         if q < 3 else [])
                fillC = c_items(q - 1, [0, 1, 2, 3]) if q > 0 else []
                pts = [[], []]
                pv0 = [lambda qc=qc: emit_pv(q, 0, pts[0], [qc])
                       for qc in range(4)]
                s0 = [lambda i=i: emit_s_tile(q, 0, i, pts[0])
                      for i in range(4 * (q + 1))]
                s1 = [lambda i=i: emit_s_tile(q, 1, i, pts[1])
                      for i in range(4 * (q + 1))]
                # phase 1: S(q,0) + C(q-1) half; phase 2: S(q,1) + C half;
                # then PV(q,0), the A(q+1) segment (so exp(q,1) drains),
                # then PV(q,1)
                spread(s0, fillC[0:4])
                spread(s1, fillC[4:8])
                for it in pv0:
                    it()
                for it in fillA:
                    it()
                if q < 3:
                    emit_pv(q, 1, pts[1], [0, 1, 2, 3])
                else:
                    # tail: stagger the final out-projection one q-tile
                    # behind PV so the transpose chain latency is hidden
                    cpend = []
                    for qc in range(4):
                        emit_pv(3, 1, pts[1], [qc])
                        if qc >= 1:
                            cpend.extend(c_items(3, [qc - 1]))
                        if len(cpend) >= 2:
                            cpend.pop(0)()
                            cpend.pop(0)()
                    for it in cpend + c_items(3, [3]):
                        it()

    nc.compile()
    return nc


def _get_nc():
    if "nc" not in _CACHE:
        _CACHE["nc"] = _build()
    return _CACHE["nc"]


def _get_runner():
    """Build the jitted SPMD executor once (mirrors bass2jax.run_bass_via_pjrt
    but caches the jitted function so repeat calls skip retrace/recompile)."""
    if "runner" in _CACHE:
        return _CACHE["runner"]
    import jax
    import numpy as _np
    from jax.sharding import Mesh, PartitionSpec
    from jax.experimental.shard_map import shard_map
    import concourse.mybir as mybir
    from concourse import bass2jax

    nc = _get_nc()
    bass2jax.install_neuronx_cc_hook()

    partition_name = (nc.partition_id_tensor.name
                      if nc.partition_id_tensor else None)
    in_names, out_names, out_avals, zero_shapes = [], [], [], []
    for alloc in nc.m.functions[0].allocations:
        if not isinstance(alloc, mybir.MemoryLocationSet):
            continue
        name = alloc.memorylocations[0].name
        if alloc.kind == "ExternalInput":
            if name != partition_name:
                in_names.append(name)
        elif alloc.kind == "ExternalOutput":
            out_avals.append(jax.core.ShapedArray(
                tuple(alloc.tensor_shape), mybir.dt.np(alloc.dtype)))
            out_names.append(name)
            zero_shapes.append((tuple(alloc.tensor_shape),
                                mybir.dt.np(alloc.dtype)))
    n_params = len(in_names)
    n_outs = len(out_names)
    all_names = in_names + out_names
    if partition_name is not None:
        all_names = all_names + [partition_name]

    def _body(*args):
        operands = list(args)
        if partition_name is not None:
            operands.append(bass2jax.partition_id_tensor())
        outs = bass2jax._bass_exec_p.bind(
            *operands,
            out_avals=tuple(out_avals),
            in_names=tuple(all_names),
            out_names=tuple(out_names),
            lowering_input_output_aliases=(),
            sim_require_finite=True,
            sim_require_nnan=True,
            nc=nc,
        )
        return tuple(outs)

    devices = jax.devices()[:N_CORES]
    mesh = Mesh(_np.asarray(devices), ("core",))
    donate = tuple(range(n_params, n_params + n_outs))
    sharded = jax.jit(
        shard_map(_body, mesh=mesh,
                  in_specs=(PartitionSpec("core"),) * (n_params + n_outs),
                  out_specs=(PartitionSpec("core"),) * n_outs,
                  check_rep=False),
        donate_argnums=donate, keep_unused=True)

    def run(in_maps):
        concat_in = [
            _np.concatenate([_np.asarray(m[name]) for m in in_maps], axis=0)
            for name in in_names]
        concat_zeros = [
            _np.zeros((N_CORES * sh[0], *sh[1:]), dtype)
            for sh, dtype in zero_shapes]
        out_arrs = sharded(*concat_in, *concat_zeros)
        return [
            {name: _np.asarray(out_arrs[i]).reshape(
                N_CORES, *zero_shapes[i][0])[c]
             for i, name in enumerate(out_names)}
            for c in range(N_CORES)]

    _CACHE["runner"] = run
    return run


def kernel(x, mask, W_qkv, b_qkv, W_out, b_out):
    bf = ml_dtypes.bfloat16
    x = np.asarray(x, dtype=np.float32)
    W_qkv = np.asarray(W_qkv, dtype=np.float32)
    b_qkv = np.asarray(b_qkv, dtype=np.float32)
    W_out = np.asarray(W_out, dtype=np.float32)
    b_out = np.asarray(b_out, dtype=np.float32)
    # mask is the causal tril mask (per problem spec); causality is
    # implemented structurally on-device.

    run = _get_runner()

    def pack_w(wslice):
        # [C, DH] -> [P, KO, DH] with C = ko*P + p
        return np.ascontiguousarray(
            wslice.reshape(KO, P, DH).transpose(1, 0, 2)).astype(bf)

    def pack_b(bslice):
        # [DH] -> [P, 2] with idx = s*P + p
        return np.ascontiguousarray(
            bslice.reshape(2, P).T).astype(np.float32)

    tri = np.triu(np.ones((P, P), dtype=np.float32)).astype(bf)

    xts = [np.ascontiguousarray(x[b].T).astype(bf) for b in range(B)]
    in_maps = []
    for core in range(N_CORES):
        b, g = divmod(core, GROUPS)
        cs = slice(g * DH, (g + 1) * DH)
        in_maps.append({
            "xt": xts[b],
            "wq": pack_w(W_qkv[:, cs]),
            "wk": pack_w(W_qkv[:, C:][:, cs]),
            "wv": pack_w(W_qkv[:, 2 * C:][:, cs]),
            "wo": np.ascontiguousarray(
                W_out[cs, :].reshape(2, P, C).transpose(1, 0, 2)).astype(bf),
            "bq": pack_b(b_qkv[cs]),
            "bk": pack_b(b_qkv[C:][cs]),
            "bv": np.ascontiguousarray(
                b_qkv[2 * C:][cs][None, :]).astype(np.float32),
            "tri": tri,
        })

    results = run(in_maps)

    out = np.zeros((B, T, C), dtype=np.float32)
    for core in range(N_CORES):
        b = core // GROUPS
        out[b] += results[core]["out"].astype(np.float32)
    out += b_out[None, None, :]
    return out


# revision 18
# speedup vs baseline: 1.0167x; 1.0167x over previous
"""Trainium2 Bass kernel for nn_CausalSelfAttention (B=2, T=2048, C=1024, 16 heads).

Sharding: 8 cores = 2 batches x 4 head-groups (4 heads each). Each core:
  - computes Q^T/K^T ([d,t] layout) and V ([t,d]) for its heads from x^T
    (host pre-transposes x and pre-packs the weight slices in SBUF layout),
  - runs causal flash attention: S^T ([k,q]) via PE, exp on ScalarE with the
    1/sqrt(d) scale fused, then P@V with P^T as the *stationary* operand so
    the PV matmul runs at full PE rate ([q,d] output, 128-partition out),
    softmax denominators from a ones-column in V,
  - normalizes O in [q,d] layout (per-partition scalars), transposes O back
    to [d,q] via the DMA XBAR (off the PE critical path),
  - projects through its W_out row-slice producing a partial [T, C] output.
Out-projection blocks are interleaved into the attention loop as PE fillers;
a single set of tile pools is used throughout (no mid-kernel pool barriers).
Host sums the 4 tensor-parallel partials per batch (the "all-reduce") and
adds b_out.

Matmul inputs are bf16 (fp32 accumulation in PSUM).
"""
import sys

if '/opt/trn_rl_repo' not in sys.path:
    sys.path.insert(0, '/opt/trn_rl_repo')

import numpy as np
import ml_dtypes

B, T, C = 2, 2048, 1024
N_HEAD = 16
D = 64
P = 128
N_CORES = 8
GROUPS = N_CORES // B            # 4 tensor-parallel groups per batch
HPC = N_HEAD // GROUPS           # 4 heads per core
DH = HPC * D                     # 256 head dims per core
KO = C // P                      # 8 contraction subtiles for projections
NQB = T // 512                   # 4 q blocks of 512
SCALE = 1.0 / np.sqrt(D)
N_WARM = 12                       # PE warm-up matmuls while first loads land

_CACHE = {}


def _build():
    import concourse.mybir as mybir
    import concourse.tile as tile
    from concourse import bacc

    f32 = mybir.dt.float32
    bf16 = mybir.dt.bfloat16
    f16 = mybir.dt.float16

    nc = bacc.Bacc("TRN2", target_bir_lowering=False, debug=False,
                   num_devices=N_CORES)

    xt_d = nc.dram_tensor("xt", [C, T], bf16, kind="ExternalInput")
    wq_d = nc.dram_tensor("wq", [P, KO, DH], bf16, kind="ExternalInput")
    wk_d = nc.dram_tensor("wk", [P, KO, DH], bf16, kind="ExternalInput")
    wv_d = nc.dram_tensor("wv", [P, KO, DH], bf16, kind="ExternalInput")
    wo_d = nc.dram_tensor("wo", [P, 2, C], bf16, kind="ExternalInput")
    bq_d = nc.dram_tensor("bq", [P, 2], f32, kind="ExternalInput")
    bk_d = nc.dram_tensor("bk", [P, 2], f32, kind="ExternalInput")
    bv_d = nc.dram_tensor("bv", [1, DH], f32, kind="ExternalInput")
    tri_d = nc.dram_tensor("tri", [P, P], bf16, kind="ExternalInput")
    out_d = nc.dram_tensor("out", [T, C], f16, kind="ExternalOutput")

    EXP = mybir.ActivationFunctionType.Exp
    pls = [slice(0, D), slice(D, 2 * D)]

    with tile.TileContext(nc) as tc:
        with (
            tc.tile_pool(name="pp", bufs=1) as pp,
            tc.tile_pool(name="wB", bufs=32) as wB,
            tc.tile_pool(name="wS", bufs=4) as wS,
            tc.tile_pool(name="wO", bufs=4) as wO,
            tc.tile_pool(name="psS", bufs=2, space="PSUM") as psS,
            tc.tile_pool(name="psU", bufs=4, space="PSUM") as psU,
        ):
            xts = [pp.tile([P, KO, 512], bf16, tag=f"xt{q}", name=f"xt{q}")
                   for q in range(4)]
            wqs = pp.tile([P, KO, DH], bf16, tag="wqs")
            wks = pp.tile([P, KO, DH], bf16, tag="wks")
            wvs = pp.tile([P, KO, DH], bf16, tag="wvs")
            wos = pp.tile([P, 2, C], bf16, tag="wos")
            qts = [[pp.tile([P, 512], bf16, tag=f"qt{s}_{q}", name=f"qt{s}_{q}")
                    for q in range(4)] for s in range(2)]
            kts = [[pp.tile([P, 512], bf16, tag=f"kt{s}_{q}", name=f"kt{s}_{q}")
                    for q in range(4)] for s in range(2)]
            vos = [pp.tile([P, 4, HPC, D + 1], bf16, tag=f"vo{q}",
                           name=f"vo{q}") for q in range(4)]
            ots = [[pp.tile([P, 512], bf16, tag=f"ot{j}_{hs}",
                            name=f"ot{j}_{hs}") for hs in range(2)]
                   for j in range(NQB)]
            bqs = pp.tile([P, 2], f32, tag="bqs")
            bks = pp.tile([P, 2], f32, tag="bks")
            bvrow = pp.tile([1, DH], f32, tag="bvrow")
            bvb = pp.tile([P, DH], f32, tag="bvb")
            trib = pp.tile([P, P], bf16, tag="trib")
            warm = pp.tile([P, 512], bf16, tag="warm")

            # PE warm-up: junk matmuls on a zeroed tile keep the PE p-state
            # ramp going while the first input DMAs land.
            nc.vector.memset(warm[:], 0.0)
            for w in range(N_WARM):
                pw = psU.tile([P, 512], f32, tag="ps1", name=f"pw{w}")
                nc.tensor.matmul(pw[:], warm[:, 0:P], warm[:],
                                 start=True, stop=True)

            # ---- loads (SP + ACT HWDGE queues in parallel at startup) ----
            xt_r = xt_d.rearrange("(ko p) t -> p ko t", p=P)
            nc.sync.dma_start(wqs[:, :, 0:P], wq_d[:, :, 0:P])
            nc.scalar.dma_start(xts[0][:, :, 0:256], xt_r[:, :, 0:256])
            nc.sync.dma_start(bqs[:], bq_d[:])
            nc.sync.dma_start(wqs[:, :, P:2 * P], wq_d[:, :, P:2 * P])
            nc.scalar.dma_start(xts[0][:, :, 256:512], xt_r[:, :, 256:512])
            nc.sync.dma_start(bks[:], bk_d[:])
            nc.sync.dma_start(trib[:], tri_d[:])
            nc.sync.dma_start(bvrow[:], bv_d[:])
            nc.sync.dma_start(wks[:], wk_d[:])
            nc.sync.dma_start(xts[1][:], xt_r[:, :, 512:1024])
            nc.sync.dma_start(wvs[:], wv_d[:])
            nc.sync.dma_start(xts[2][:], xt_r[:, :, 1024:1536])
            nc.sync.dma_start(xts[3][:], xt_r[:, :, 1536:2048])
            nc.sync.dma_start(wos[:], wo_d[:])

            nc.gpsimd.partition_broadcast(bvb[:, :], bvrow[0:1, :])
            for q in range(4):
                nc.vector.memset(vos[q][:, :, :, D:D + 1], 1.0)
            # trigger the exp ACT-table load early
            scr = pp.tile([1, 1], f32, tag="scr")
            nc.scalar.activation(scr[0:1, 0:1], trib[0:1, 0:1], EXP)

            # ---- emit helpers (each *_items returns a list of thunks; one
            # thunk = one contiguous chunk of PE work) ----
            def qk_items(q, chunks):
                items = []
                for wsb, dsts, bias in ((wqs, qts, bqs), (wks, kts, bks)):
                    for s_ in range(2):
                        for (lo, hi) in chunks:
                            def it(wsb=wsb, dsts=dsts, bias=bias, s_=s_,
                                   lo=lo, hi=hi):
                                w_ = hi - lo
                                pq = psU.tile([P, 512], f32, tag="ps1",
                                              name=f"pq{q}_{s_}_{lo}")
                                for ko in range(KO):
                                    nc.tensor.matmul(
                                        pq[:, 0:w_],
                                        wsb[:, ko, s_ * P:(s_ + 1) * P],
                                        xts[q][:, ko, lo:hi],
                                        start=(ko == 0), stop=(ko == KO - 1))
                                nc.vector.tensor_scalar_add(
                                    dsts[s_][q][:, lo:hi], pq[:, 0:w_],
                                    bias[:, s_:s_ + 1])
                            items.append(it)
                return items

            def v_items(q):
                items = []
                for it_ in range(4):
                    def it(it_=it_):
                        pv = psU.tile([P, 512], f32, tag="ps1",
                                      name=f"pv{q}_{it_}")
                        for ko in range(KO):
                            nc.tensor.matmul(
                                pv[:, 0:DH],
                                xts[q][:, ko, it_ * P:(it_ + 1) * P],
                                wvs[:, ko, :],
                                start=(ko == 0), stop=(ko == KO - 1))
                        nc.vector.tensor_tensor(
                            vos[q][:, it_, :, 0:D],
                            pv[:, 0:DH].rearrange("p (h d) -> p h d", h=HPC),
                            bvb.rearrange("p (h d) -> p h d", h=HPC),
                            mybir.AluOpType.add)
                    items.append(it)
                return items

            def emit_s_tile(q, hs, i, pts):
                off = max(0, P * i - 512 * q)
                sp = psS.tile([P, 2, 512], f32, tag="sp",
                              name=f"sp{q}_{hs}_{i}")
                pt = wB.tile([P, 2, 512], bf16, tag="pt",
                             name=f"pt{q}_{hs}_{i}")
                for u in range(2):
                    nc.tensor.matmul(
                        sp[:, u, off:512],
                        kts[hs][i // 4][pls[u],
                                        (i % 4) * P:(i % 4 + 1) * P],
                        qts[hs][q][pls[u], off:512],
                        start=True, stop=True)
                nc.scalar.activation(pt[:, :, off:512],
                                     sp[:, :, off:512],
                                     EXP, scale=float(SCALE))
                if P * i >= 512 * q:  # diagonal triangle
                    for u in range(2):
                        nc.vector.tensor_mul(
                            pt[:, u, off:off + P],
                            pt[:, u, off:off + P], trib[:])
                pts.append(pt)

            def emit_pv(q, hs, pts, qcs):
                for qc in qcs:
                    qt = 4 * q + qc
                    # one PSUM bank per head (zero-region = 2KB: only one
                    # accumulation group may live in a bank at a time)
                    po = [psU.tile([P, 512], f32, tag="ps1",
                                   name=f"po{q}_{hs}_{qc}_{u}")
                          for u in range(2)]
                    for u in range(2):
                        for kt in range(qt + 1):
                            nc.tensor.matmul(
                                po[u][:, 0:D + 1],
                                pts[kt][:, u, qc * P:(qc + 1) * P],
                                vos[kt // 4][:, kt % 4, 2 * hs + u, :],
                                start=(kt == 0), stop=(kt == qt))
                    rd = wS.tile([P, 2], f32, tag="rd",
                                 name=f"rd{q}_{hs}_{qc}")
                    osb = wS.tile([P, 2, D], bf16, tag="osb",
                                  name=f"osb{q}_{hs}_{qc}")
                    for u in range(2):
                        nc.vector.reciprocal_approx_fast(
                            rd[:, u:u + 1], po[u][:, D:D + 1])
                        nc.vector.tensor_scalar_mul(
                            osb[:, u, :], po[u][:, 0:D], rd[:, u:u + 1])
                    nc.sync.dma_start(
                        ots[q][hs][:, qc * P:(qc + 1) * P], osb[:],
                        transpose=True)

            def c_items(jj, mos):
                # one [P, C] staging tile + ONE store per row-block, issued
                # through the software DGE on the idle GpSimd engine so the
                # stores never contend with loads/transposes on HWDGE
                items = []
                obs = {}
                for mo in mos:
                    for n in range(2):
                        def it(mo=mo, n=n):
                            m = 4 * jj + mo
                            pc = psU.tile([P, 512], f32, tag="ps1",
                                          name=f"pc{jj}_{mo}_{n}")
                            for s in range(2):
                                nc.tensor.matmul(
                                    pc[:],
                                    ots[jj][s][:, mo * P:(mo + 1) * P],
                                    wos[:, s, n * 512:(n + 1) * 512],
                                    start=(s == 0), stop=(s == 1))
                            if n == 0:
                                obs[mo] = wO.tile([P, C], f16, tag="ob",
                                                  name=f"ob{jj}_{mo}")
                            ob = obs[mo]
                            nc.vector.tensor_copy(
                                ob[:, n * 512:(n + 1) * 512], pc[:])
                            if n == 1:
                                nc.gpsimd.dma_start(
                                    out_d[m * P:(m + 1) * P, :], ob[:])
                        items.append(it)
                return items

            # ---- main pipeline ----
            # Per iteration q: spread the S i-tiles (whose exps are the ACT
            # bottleneck) among filler PE work (prev block's out-projection,
            # part of the next quarter's projections); the rest of the A
            # segment sits between PV(q,0) and PV(q,1) so the second head
            # pair's exps can drain before PV(q,1) consumes them.
            def spread(tiles, fillers):
                # emit S tiles with fillers distributed evenly between them
                nS, nF = len(tiles), len(fillers)
                fi = 0
                for k, t in enumerate(tiles, 1):
                    t()
                    while fi < (k * nF) // nS:
                        fillers[fi]()
                        fi += 1

            for it in qk_items(0, [(0, 256), (256, 512)]) + v_items(0):
                it()
            for q in range(4):
                fillA = (qk_items(q + 1, [(0, 512)]) + v_items(q + 1)
                         if q < 3 else [])
                fillC = c_items(q - 1, [0, 1, 2, 3]) if q > 0 else []
                pts = [[], []]
                pv0 = [lambda qc=qc: emit_pv(q, 0, pts[0], [qc])
                       for qc in range(4)]
                s0 = [lambda i=i: emit_s_tile(q, 0, i, pts[0])
                      for i in range(4 * (q + 1))]
                s1 = [lambda i=i: emit_s_tile(q, 1, i, pts[1])
                      for i in range(4 * (q + 1))]
                # S tiles spread with C(q-1) + part of A(q+1) as fillers;
                # then PV(q,0), the rest of the A segment (so exp(q,1)
                # drains), then PV(q,1)
                spread(s0 + s1, fillC + fillA[0:4])
                for it in pv0:
                    it()
                for it in fillA[4:]:
                    it()
                if q < 3:
                    emit_pv(q, 1, pts[1], [0, 1, 2, 3])
                else:
                    # tail: stagger the final out-projection one q-tile
                    # behind PV so the transpose chain latency is hidden
                    cpend = []
                    for qc in range(4):
                        emit_pv(3, 1, pts[1], [qc])
                        if qc >= 1:
                            cpend.extend(c_items(3, [qc - 1]))
                        if len(cpend) >= 2:
                            cpend.pop(0)()
                            cpend.pop(0)()
                    for it in cpend + c_items(3, [3]):
                        it()

    nc.compile()
    return nc


def _get_nc():
    if "nc" not in _CACHE:
        _CACHE["nc"] = _build()
    return _CACHE["nc"]


def _get_runner():
    """Build the jitted SPMD executor once (mirrors bass2jax.run_bass_via_pjrt
    but caches the jitted function so repeat calls skip retrace/recompile)."""
    if "runner" in _CACHE:
        return _CACHE["runner"]
    import jax
    import numpy as _np
    from jax.sharding import Mesh, PartitionSpec
    from jax.experimental.shard_map import shard_map
    import concourse.mybir as mybir
    from concourse import bass2jax

    nc = _get_nc()
    bass2jax.install_neuronx_cc_hook()

    partition_name = (nc.partition_id_tensor.name
                      if nc.partition_id_tensor else None)
    in_names, out_names, out_avals, zero_shapes = [], [], [], []
    for alloc in nc.m.functions[0].allocations:
        if not isinstance(alloc, mybir.MemoryLocationSet):
            continue
        name = alloc.memorylocations[0].name
        if alloc.kind == "ExternalInput":
            if name != partition_name:
                in_names.append(name)
        elif alloc.kind == "ExternalOutput":
            out_avals.append(jax.core.ShapedArray(
                tuple(alloc.tensor_shape), mybir.dt.np(alloc.dtype)))
            out_names.append(name)
            zero_shapes.append((tuple(alloc.tensor_shape),
                                mybir.dt.np(alloc.dtype)))
    n_params = len(in_names)
    n_outs = len(out_names)
    all_names = in_names + out_names
    if partition_name is not None:
        all_names = all_names + [partition_name]

    def _body(*args):
        operands = list(args)
        if partition_name is not None:
            operands.append(bass2jax.partition_id_tensor())
        outs = bass2jax._bass_exec_p.bind(
            *operands,
            out_avals=tuple(out_avals),
            in_names=tuple(all_names),
            out_names=tuple(out_names),
            lowering_input_output_aliases=(),
            sim_require_finite=True,
            sim_require_nnan=True,
            nc=nc,
        )
        return tuple(outs)

    devices = jax.devices()[:N_CORES]
    mesh = Mesh(_np.asarray(devices), ("core",))
    donate = tuple(range(n_params, n_params + n_outs))
    sharded = jax.jit(
        shard_map(_body, mesh=mesh,
                  in_specs=(PartitionSpec("core"),) * (n_params + n_outs),
                  out_specs=(PartitionSpec("core"),) * n_outs,
                  check_rep=False),
        donate_argnums=donate, keep_unused=True)

    def run(in_maps):
        concat_in = [
            _np.concatenate([_np.asarray(m[name]) for m in in_maps], axis=0)
            for name in in_names]
        concat_zeros = [
            _np.zeros((N_CORES * sh[0], *sh[1:]), dtype)
            for sh, dtype in zero_shapes]
        out_arrs = sharded(*concat_in, *concat_zeros)
        return [
            {name: _np.asarray(out_arrs[i]).reshape(
                N_CORES, *zero_shapes[i][0])[c]
             for i, name in enumerate(out_names)}
            for c in range(N_CORES)]

    _CACHE["runner"] = run
    return run


def kernel(x, mask, W_qkv, b_qkv, W_out, b_out):
    bf = ml_dtypes.bfloat16
    x = np.asarray(x, dtype=np.float32)
    W_qkv = np.asarray(W_qkv, dtype=np.float32)
    b_qkv = np.asarray(b_qkv, dtype=np.float32)
    W_out = np.asarray(W_out, dtype=np.float32)
    b_out = np.asarray(b_out, dtype=np.float32)
    # mask is the causal tril mask (per problem spec); causality is
    # implemented structurally on-device.

    run = _get_runner()

    def pack_w(wslice):
        # [C, DH] -> [P, KO, DH] with C = ko*P + p
        return np.ascontiguousarray(
            wslice.reshape(KO, P, DH).transpose(1, 0, 2)).astype(bf)

    def pack_b(bslice):
        # [DH] -> [P, 2] with idx = s*P + p
        return np.ascontiguousarray(
            bslice.reshape(2, P).T).astype(np.float32)

    tri = np.triu(np.ones((P, P), dtype=np.float32)).astype(bf)

    xts = [np.ascontiguousarray(x[b].T).astype(bf) for b in range(B)]
    in_maps = []
    for core in range(N_CORES):
        b, g = divmod(core, GROUPS)
        cs = slice(g * DH, (g + 1) * DH)
        in_maps.append({
            "xt": xts[b],
            "wq": pack_w(W_qkv[:, cs]),
            "wk": pack_w(W_qkv[:, C:][:, cs]),
            "wv": pack_w(W_qkv[:, 2 * C:][:, cs]),
            "wo": np.ascontiguousarray(
                W_out[cs, :].reshape(2, P, C).transpose(1, 0, 2)).astype(bf),
            "bq": pack_b(b_qkv[cs]),
            "bk": pack_b(b_qkv[C:][cs]),
            "bv": np.ascontiguousarray(
                b_qkv[2 * C:][cs][None, :]).astype(np.float32),
            "tri": tri,
        })

    results = run(in_maps)

    out = np.zeros((B, T, C), dtype=np.float32)
    for core in range(N_CORES):
        b = core // GROUPS
        out[b] += results[core]["out"].astype(np.float32)
    out += b_out[None, None, :]
    return out


# revision 19
# speedup vs baseline: 1.0452x; 1.0281x over previous
"""Trainium2 Bass kernel for nn_CausalSelfAttention (B=2, T=2048, C=1024, 16 heads).

Sharding: 8 cores = 2 batches x 4 head-groups (4 heads each). Each core:
  - computes Q^T/K^T ([d,t] layout) and V ([t,d]) for its heads from x^T
    (host pre-transposes x and pre-packs the weight slices in SBUF layout),
  - runs causal flash attention: S^T ([k,q]) via PE, exp on ScalarE with the
    1/sqrt(d) scale fused, then P@V with P^T as the *stationary* operand so
    the PV matmul runs at full PE rate ([q,d] output, 128-partition out),
    softmax denominators from a ones-column in V,
  - normalizes O in [q,d] layout (per-partition scalars), transposes O back
    to [d,q] via the DMA XBAR (off the PE critical path),
  - projects through its W_out row-slice producing a partial [T, C] output.
Out-projection blocks are interleaved into the attention loop as PE fillers;
a single set of tile pools is used throughout (no mid-kernel pool barriers).
Host sums the 4 tensor-parallel partials per batch (the "all-reduce") and
adds b_out.

Matmul inputs are bf16 (fp32 accumulation in PSUM).
"""
import sys

if '/opt/trn_rl_repo' not in sys.path:
    sys.path.insert(0, '/opt/trn_rl_repo')

import numpy as np
import ml_dtypes

B, T, C = 2, 2048, 1024
N_HEAD = 16
D = 64
P = 128
N_CORES = 8
GROUPS = N_CORES // B            # 4 tensor-parallel groups per batch
HPC = N_HEAD // GROUPS           # 4 heads per core
DH = HPC * D                     # 256 head dims per core
KO = C // P                      # 8 contraction subtiles for projections
NQB = T // 512                   # 4 q blocks of 512
SCALE = 1.0 / np.sqrt(D)
N_WARM = 12                       # PE warm-up matmuls while first loads land

_CACHE = {}


def _build():
    import concourse.mybir as mybir
    import concourse.tile as tile
    from concourse import bacc

    f32 = mybir.dt.float32
    bf16 = mybir.dt.bfloat16
    f16 = mybir.dt.float16

    nc = bacc.Bacc("TRN2", target_bir_lowering=False, debug=False,
                   num_devices=N_CORES)

    xt_d = nc.dram_tensor("xt", [C, T], bf16, kind="ExternalInput")
    wq_d = nc.dram_tensor("wq", [P, KO, DH], bf16, kind="ExternalInput")
    wk_d = nc.dram_tensor("wk", [P, KO, DH], bf16, kind="ExternalInput")
    wv_d = nc.dram_tensor("wv", [P, KO, DH], bf16, kind="ExternalInput")
    wo_d = nc.dram_tensor("wo", [P, 2, C], bf16, kind="ExternalInput")
    bq_d = nc.dram_tensor("bq", [P, 2], f32, kind="ExternalInput")
    bk_d = nc.dram_tensor("bk", [P, 2], f32, kind="ExternalInput")
    bv_d = nc.dram_tensor("bv", [1, DH], f32, kind="ExternalInput")
    tri_d = nc.dram_tensor("tri", [P, P], bf16, kind="ExternalInput")
    out_d = nc.dram_tensor("out", [T, C], f16, kind="ExternalOutput")

    EXP = mybir.ActivationFunctionType.Exp
    pls = [slice(0, D), slice(D, 2 * D)]

    with tile.TileContext(nc) as tc:
        with (
            tc.tile_pool(name="pp", bufs=1) as pp,
            tc.tile_pool(name="wB", bufs=32) as wB,
            tc.tile_pool(name="wS", bufs=4) as wS,
            tc.tile_pool(name="wO", bufs=4) as wO,
            tc.tile_pool(name="psS", bufs=2, space="PSUM") as psS,
            tc.tile_pool(name="psU", bufs=4, space="PSUM") as psU,
        ):
            xts = [pp.tile([P, KO, 512], bf16, tag=f"xt{q}", name=f"xt{q}")
                   for q in range(4)]
            wqs = pp.tile([P, KO, DH], bf16, tag="wqs")
            wks = pp.tile([P, KO, DH], bf16, tag="wks")
            wvs = pp.tile([P, KO, DH], bf16, tag="wvs")
            wos = pp.tile([P, 2, C], bf16, tag="wos")
            qts = [[pp.tile([P, 512], bf16, tag=f"qt{s}_{q}", name=f"qt{s}_{q}")
                    for q in range(4)] for s in range(2)]
            kts = [[pp.tile([P, 512], bf16, tag=f"kt{s}_{q}", name=f"kt{s}_{q}")
                    for q in range(4)] for s in range(2)]
            vos = [pp.tile([P, 4, HPC, D + 1], bf16, tag=f"vo{q}",
                           name=f"vo{q}") for q in range(4)]
            ots = [[pp.tile([P, 512], bf16, tag=f"ot{j}_{hs}",
                            name=f"ot{j}_{hs}") for hs in range(2)]
                   for j in range(NQB)]
            bqs = pp.tile([P, 2], f32, tag="bqs")
            bks = pp.tile([P, 2], f32, tag="bks")
            bvrow = pp.tile([1, DH], f32, tag="bvrow")
            bvb = pp.tile([P, DH], f32, tag="bvb")
            trib = pp.tile([P, P], bf16, tag="trib")
            warm = pp.tile([P, 512], bf16, tag="warm")

            # PE warm-up: junk matmuls on a zeroed tile keep the PE p-state
            # ramp going while the first input DMAs land.
            nc.vector.memset(warm[:], 0.0)
            for w in range(N_WARM):
                pw = psU.tile([P, 512], f32, tag="ps1", name=f"pw{w}")
                nc.tensor.matmul(pw[:], warm[:, 0:P], warm[:],
                                 start=True, stop=True)

            # ---- loads (SP + ACT HWDGE queues in parallel at startup) ----
            xt_r = xt_d.rearrange("(ko p) t -> p ko t", p=P)
            nc.sync.dma_start(wqs[:, :, 0:P], wq_d[:, :, 0:P])
            nc.scalar.dma_start(xts[0][:, :, 0:256], xt_r[:, :, 0:256])
            nc.sync.dma_start(bqs[:], bq_d[:])
            nc.sync.dma_start(wqs[:, :, P:2 * P], wq_d[:, :, P:2 * P])
            nc.scalar.dma_start(xts[0][:, :, 256:512], xt_r[:, :, 256:512])
            nc.sync.dma_start(bks[:], bk_d[:])
            nc.sync.dma_start(trib[:], tri_d[:])
            nc.sync.dma_start(bvrow[:], bv_d[:])
            nc.sync.dma_start(wks[:], wk_d[:])
            nc.sync.dma_start(xts[1][:], xt_r[:, :, 512:1024])
            nc.sync.dma_start(wvs[:], wv_d[:])
            nc.sync.dma_start(xts[2][:], xt_r[:, :, 1024:1536])
            nc.sync.dma_start(xts[3][:], xt_r[:, :, 1536:2048])
            nc.sync.dma_start(wos[:], wo_d[:])

            nc.gpsimd.partition_broadcast(bvb[:, :], bvrow[0:1, :])
            for q in range(4):
                nc.vector.memset(vos[q][:, :, :, D:D + 1], 1.0)
            # trigger the exp ACT-table load early
            scr = pp.tile([1, 1], f32, tag="scr")
            nc.scalar.activation(scr[0:1, 0:1], trib[0:1, 0:1], EXP)

            # ---- emit helpers (each *_items returns a list of thunks; one
            # thunk = one contiguous chunk of PE work) ----
            def qk_items(q, chunks):
                items = []
                for wsb, dsts, bias in ((wqs, qts, bqs), (wks, kts, bks)):
                    for s_ in range(2):
                        for (lo, hi) in chunks:
                            def it(wsb=wsb, dsts=dsts, bias=bias, s_=s_,
                                   lo=lo, hi=hi):
                                w_ = hi - lo
                                pq = psU.tile([P, 512], f32, tag="ps1",
                                              name=f"pq{q}_{s_}_{lo}")
                                for ko in range(KO):
                                    nc.tensor.matmul(
                                        pq[:, 0:w_],
                                        wsb[:, ko, s_ * P:(s_ + 1) * P],
                                        xts[q][:, ko, lo:hi],
                                        start=(ko == 0), stop=(ko == KO - 1))
                                nc.vector.tensor_scalar_add(
                                    dsts[s_][q][:, lo:hi], pq[:, 0:w_],
                                    bias[:, s_:s_ + 1])
                            items.append(it)
                return items

            def v_items(q):
                items = []
                for it_ in range(4):
                    def it(it_=it_):
                        pv = psU.tile([P, 512], f32, tag="ps1",
                                      name=f"pv{q}_{it_}")
                        for ko in range(KO):
                            nc.tensor.matmul(
                                pv[:, 0:DH],
                                xts[q][:, ko, it_ * P:(it_ + 1) * P],
                                wvs[:, ko, :],
                                start=(ko == 0), stop=(ko == KO - 1))
                        nc.vector.tensor_tensor(
                            vos[q][:, it_, :, 0:D],
                            pv[:, 0:DH].rearrange("p (h d) -> p h d", h=HPC),
                            bvb.rearrange("p (h d) -> p h d", h=HPC),
                            mybir.AluOpType.add)
                    items.append(it)
                return items

            def emit_s_tile(q, hs, i, pts):
                off = max(0, P * i - 512 * q)
                sp = psS.tile([P, 2, 512], f32, tag="sp",
                              name=f"sp{q}_{hs}_{i}")
                pt = wB.tile([P, 2, 512], bf16, tag="pt",
                             name=f"pt{q}_{hs}_{i}")
                for u in range(2):
                    nc.tensor.matmul(
                        sp[:, u, off:512],
                        kts[hs][i // 4][pls[u],
                                        (i % 4) * P:(i % 4 + 1) * P],
                        qts[hs][q][pls[u], off:512],
                        start=True, stop=True)
                nc.scalar.activation(pt[:, :, off:512],
                                     sp[:, :, off:512],
                                     EXP, scale=float(SCALE))
                if P * i >= 512 * q:  # diagonal triangle
                    for u in range(2):
                        nc.vector.tensor_mul(
                            pt[:, u, off:off + P],
                            pt[:, u, off:off + P], trib[:])
                pts.append(pt)

            def emit_pv(q, hs, pts, qcs):
                for qc in qcs:
                    qt = 4 * q + qc
                    # one PSUM bank per head (zero-region = 2KB: only one
                    # accumulation group may live in a bank at a time)
                    po = [psU.tile([P, 512], f32, tag="ps1",
                                   name=f"po{q}_{hs}_{qc}_{u}")
                          for u in range(2)]
                    for u in range(2):
                        for kt in range(qt + 1):
                            nc.tensor.matmul(
                                po[u][:, 0:D + 1],
                                pts[kt][:, u, qc * P:(qc + 1) * P],
                                vos[kt // 4][:, kt % 4, 2 * hs + u, :],
                                start=(kt == 0), stop=(kt == qt))
                    rd = wS.tile([P, 2], f32, tag="rd",
                                 name=f"rd{q}_{hs}_{qc}")
                    osb = wS.tile([P, 2, D], bf16, tag="osb",
                                  name=f"osb{q}_{hs}_{qc}")
                    for u in range(2):
                        nc.vector.reciprocal_approx_fast(
                            rd[:, u:u + 1], po[u][:, D:D + 1])
                        nc.vector.tensor_scalar_mul(
                            osb[:, u, :], po[u][:, 0:D], rd[:, u:u + 1])
                    nc.sync.dma_start(
                        ots[q][hs][:, qc * P:(qc + 1) * P], osb[:],
                        transpose=True)

            def c_items(jj, mos):
                # one [P, C] staging tile + ONE store per row-block, issued
                # through the software DGE on the idle GpSimd engine so the
                # stores never contend with loads/transposes on HWDGE
                items = []
                obs = {}
                for mo in mos:
                    for n in range(2):
                        def it(mo=mo, n=n):
                            m = 4 * jj + mo
                            pc = psU.tile([P, 512], f32, tag="ps1",
                                          name=f"pc{jj}_{mo}_{n}")
                            for s in range(2):
                                nc.tensor.matmul(
                                    pc[:],
                                    ots[jj][s][:, mo * P:(mo + 1) * P],
                                    wos[:, s, n * 512:(n + 1) * 512],
                                    start=(s == 0), stop=(s == 1))
                            if n == 0:
                                obs[mo] = wO.tile([P, C], f16, tag="ob",
                                                  name=f"ob{jj}_{mo}")
                            ob = obs[mo]
                            nc.vector.tensor_copy(
                                ob[:, n * 512:(n + 1) * 512], pc[:])
                            if n == 1:
                                nc.gpsimd.dma_start(
                                    out_d[m * P:(m + 1) * P, :], ob[:])
                        items.append(it)
                return items

            # ---- main pipeline ----
            # Per iteration q: spread the S i-tiles (whose exps are the ACT
            # bottleneck) among filler PE work (prev block's out-projection,
            # part of the next quarter's projections); the rest of the A
            # segment sits between PV(q,0) and PV(q,1) so the second head
            # pair's exps can drain before PV(q,1) consumes them.
            def spread(tiles, fillers):
                # emit S tiles with fillers distributed evenly between them
                nS, nF = len(tiles), len(fillers)
                fi = 0
                for k, t in enumerate(tiles, 1):
                    t()
                    while fi < (k * nF) // nS:
                        fillers[fi]()
                        fi += 1

            for it in qk_items(0, [(0, 256), (256, 512)]) + v_items(0):
                it()
            carry = []
            for q in range(4):
                fillA = (qk_items(q + 1, [(0, 512)]) + v_items(q + 1)
                         if q < 3 else [])
                if q == 2:
                    # V(3) moves to iteration 3's filler pool where the ACT
                    # exp backlog is binding; exp(2,1) has enough drain time
                    # without a mid segment here.
                    carry = fillA[4:]
                    fillA = fillA[0:4] + []
                fillC = c_items(q - 1, [0, 1, 2, 3]) if q > 0 else []
                if q == 3:
                    fillC = fillC + carry
                pts = [[], []]
                pv0 = [lambda qc=qc: emit_pv(q, 0, pts[0], [qc])
                       for qc in range(4)]
                s0 = [lambda i=i: emit_s_tile(q, 0, i, pts[0])
                      for i in range(4 * (q + 1))]
                s1 = [lambda i=i: emit_s_tile(q, 1, i, pts[1])
                      for i in range(4 * (q + 1))]
                # S tiles spread with C(q-1) + part of A(q+1) as fillers;
                # then PV(q,0), the rest of the A segment (so exp(q,1)
                # drains), then PV(q,1)
                spread(s0 + s1, fillC + fillA[0:4])
                for it in pv0:
                    it()
                for it in fillA[4:]:
                    it()
                if q < 3:
                    emit_pv(q, 1, pts[1], [0, 1, 2, 3])
                else:
                    # tail: stagger the final out-projection one q-tile
                    # behind PV so the transpose chain latency is hidden
                    cpend = []
                    for qc in range(4):
                        emit_pv(3, 1, pts[1], [qc])
                        if qc >= 1:
                            cpend.extend(c_items(3, [qc - 1]))
                        if len(cpend) >= 2:
                            cpend.pop(0)()
                            cpend.pop(0)()
                    for it in cpend + c_items(3, [3]):
                        it()

    nc.compile()
    return nc


def _get_nc():
    if "nc" not in _CACHE:
        _CACHE["nc"] = _build()
    return _CACHE["nc"]


def _get_runner():
    """Build the jitted SPMD executor once (mirrors bass2jax.run_bass_via_pjrt
    but caches the jitted function so repeat calls skip retrace/recompile)."""
    if "runner" in _CACHE:
        return _CACHE["runner"]
    import jax
    import numpy as _np
    from jax.sharding import Mesh, PartitionSpec
    from jax.experimental.shard_map import shard_map
    import concourse.mybir as mybir
    from concourse import bass2jax

    nc = _get_nc()
    bass2jax.install_neuronx_cc_hook()

    partition_name = (nc.partition_id_tensor.name
                      if nc.partition_id_tensor else None)
    in_names, out_names, out_avals, zero_shapes = [], [], [], []
    for alloc in nc.m.functions[0].allocations:
        if not isinstance(alloc, mybir.MemoryLocationSet):
            continue
        name = alloc.memorylocations[0].name
        if alloc.kind == "ExternalInput":
            if name != partition_name:
                in_names.append(name)
        elif alloc.kind == "ExternalOutput":
            out_avals.append(jax.core.ShapedArray(
                tuple(alloc.tensor_shape), mybir.dt.np(alloc.dtype)))
            out_names.append(name)
            zero_shapes.append((tuple(alloc.tensor_shape),
                                mybir.dt.np(alloc.dtype)))
    n_params = len(in_names)
    n_outs = len(out_names)
    all_names = in_names + out_names
    if partition_name is not None:
        all_names = all_names + [partition_name]

    def _body(*args):
        operands = list(args)
        if partition_name is not None:
            operands.append(bass2jax.partition_id_tensor())
        outs = bass2jax._bass_exec_p.bind(
            *operands,
            out_avals=tuple(out_avals),
            in_names=tuple(all_names),
            out_names=tuple(out_names),
            lowering_input_output_aliases=(),
            sim_require_finite=True,
            sim_require_nnan=True,
            nc=nc,
        )
        return tuple(outs)

    devices = jax.devices()[:N_CORES]
    mesh = Mesh(_np.asarray(devices), ("core",))
    donate = tuple(range(n_params, n_params + n_outs))
    sharded = jax.jit(
        shard_map(_body, mesh=mesh,
                  in_specs=(PartitionSpec("core"),) * (n_params + n_outs),
                  out_specs=(PartitionSpec("core"),) * n_outs,
                  check_rep=False),
        donate_argnums=donate, keep_unused=True)

    def run(in_maps):
        concat_in = [
            _np.concatenate([_np.asarray(m[name]) for m in in_maps], axis=0)
            for name in in_names]
        concat_zeros = [
            _np.zeros((N_CORES * sh[0], *sh[1:]), dtype)
            for sh, dtype in zero_shapes]
        out_arrs = sharded(*concat_in, *concat_zeros)
        return [
            {name: _np.asarray(out_arrs[i]).reshape(
                N_CORES, *zero_shapes[i][0])[c]
             for i, name in enumerate(out_names)}
            for c in range(N_CORES)]

    _CACHE["runner"] = run
    return run


def kernel(x, mask, W_qkv, b_qkv, W_out, b_out):
    bf = ml_dtypes.bfloat16
    x = np.asarray(x, dtype=np.float32)
    W_qkv = np.asarray(W_qkv, dtype=np.float32)
    b_qkv = np.asarray(b_qkv, dtype=np.float32)
    W_out = np.asarray(W_out, dtype=np.float32)
    b_out = np.asarray(b_out, dtype=np.float32)
    # mask is the causal tril mask (per problem spec); causality is
    # implemented structurally on-device.

    run = _get_runner()

    def pack_w(wslice):
        # [C, DH] -> [P, KO, DH] with C = ko*P + p
        return np.ascontiguousarray(
            wslice.reshape(KO, P, DH).transpose(1, 0, 2)).astype(bf)

    def pack_b(bslice):
        # [DH] -> [P, 2] with idx = s*P + p
        return np.ascontiguousarray(
            bslice.reshape(2, P).T).astype(np.float32)

    tri = np.triu(np.ones((P, P), dtype=np.float32)).astype(bf)

    xts = [np.ascontiguousarray(x[b].T).astype(bf) for b in range(B)]
    in_maps = []
    for core in range(N_CORES):
        b, g = divmod(core, GROUPS)
        cs = slice(g * DH, (g + 1) * DH)
        in_maps.append({
            "xt": xts[b],
            "wq": pack_w(W_qkv[:, cs]),
            "wk": pack_w(W_qkv[:, C:][:, cs]),
            "wv": pack_w(W_qkv[:, 2 * C:][:, cs]),
            "wo": np.ascontiguousarray(
                W_out[cs, :].reshape(2, P, C).transpose(1, 0, 2)).astype(bf),
            "bq": pack_b(b_qkv[cs]),
            "bk": pack_b(b_qkv[C:][cs]),
            "bv": np.ascontiguousarray(
                b_qkv[2 * C:][cs][None, :]).astype(np.float32),
            "tri": tri,
        })

    results = run(in_maps)

    out = np.zeros((B, T, C), dtype=np.float32)
    for core in range(N_CORES):
        b = core // GROUPS
        out[b] += results[core]["out"].astype(np.float32)
    out += b_out[None, None, :]
    return out
